# revision 2
# baseline (speedup 1.0000x reference)
"""Trainium2 Bass kernel for nn_Net_9655086481488 (IndRNN encoder/decoder).

Mathematical reduction (exact, holds for any input values):
  - reference takes y[:, -1] after the encoder: only batch element B-1 of the
    encoder output is used.
  - it then takes out[:, 0] after the decoder, whose batch dim is the encoder
    TIME dim: only encoder timestep 0 survives.
  - the IndRNN scan starts from h0 = 0, so timestep 0 of each encoder layer is
    just relu(W @ x_0 + b) -- no recurrence needed.
  => predict depends only on v = x[0, B-1, :] (2 floats):
       h1   = relu(enc_w0 @ v + enc_b0)                  (1024,)
       h2   = relu(enc_w1 @ h1 + enc_b1)                 (1024,)
       p0   = dec_w0 @ h2 + dec_b0                       (1024,)  const over p
       g_p  = relu(p0 + dec_u0 * g_{p-1})                20-step scan
            = relu(p0) * a_p   with a_p = max(dec_u0 * a_{p-1} + 1, 0), a_0 = 1
       pre2 = G @ dec_w1.T + dec_b1                      (20, 1024)
       o_p  = relu(pre2_p + dec_u1 * o_{p-1})            20-step scan
       predict = O @ out_w.T + out_b                     (20, 2)

Sharding over 8 cores: enc_w1 / dec_w0 replicated (full vectors needed for the
nonlinear chain); dec_w1 / out_w / dec_u1 / dec_b1 sharded by 128 hidden lanes
per core. Each core returns a (20, 2) partial of the output head; the host sum
of the 8 partials (+ out_b) is the gather/unshard step. No collectives.

The three 1024x1024 weight matrices ship as float16 (the decoder scans amplify
rounding ~30x on |u|~1 lanes: bf16 lands at 2.8e-2 rel err, fp16 at 9.8e-4).
Weights are pre-transposed and pre-tiled on the host so every DMA is fully
contiguous per partition and every matmul consumes natural [K, M] layouts.
"""

import numpy as np

T = 20          # encoder timesteps
P = 20          # predict steps
B = 4096
H = 1024
NCORES = 8
HC = H // NCORES  # 128 hidden lanes per core / per chunk
KC = H // 128     # 8 k-chunks of 128
NSLAB = 4         # dma slabs per big matrix (KC % NSLAB == 0)

# small-pack column layout (one (128, 56) f32 tile per core)
C_U0 = 0      # cols 0-7:   dec_u0   chunk-major
C_EB1 = 8     # cols 8-15:  enc_b1
C_DB0 = 16    # cols 16-23: dec_b0
C_EB0 = 24    # cols 24-31: enc_b0
C_U1 = 32     # col 32:     dec_u1 shard (this core's 128 lanes)
C_DB1 = 33    # col 33:     dec_b1 shard
C_W00 = 34    # cols 34-41: enc_w0[:, 0] chunk-major
C_W01 = 42    # cols 42-49: enc_w0[:, 1] chunk-major
C_V0 = 50     # col 50:     x[0, B-1, 0] replicated
C_V1 = 51     # col 51:     x[0, B-1, 1] replicated
C_OW = 52     # cols 52-53: out_w.T shard (128, 2)
C_OWN = 54    # cols 54-55: negated out_w.T shard
NSMALL = 56

_CACHE = {}


def _build():
    import concourse.mybir as mybir
    from concourse import bacc, tile

    f32 = mybir.dt.float32
    f16 = mybir.dt.float16
    Relu = mybir.ActivationFunctionType.Relu
    mult = mybir.AluOpType.mult
    add = mybir.AluOpType.add
    sub = mybir.AluOpType.subtract
    amax = mybir.AluOpType.max

    nc = bacc.Bacc("TRN2", target_bir_lowering=False, debug=False,
                   num_devices=NCORES)

    small_h = nc.dram_tensor("small", [128, NSMALL], f32, kind="ExternalInput")
    # pre-tiled on host: [p, kc, m] = W.T[kc*128 + p, m], fully contiguous
    ew1T_h = nc.dram_tensor("ew1T", [128, KC, H], f16, kind="ExternalInput")
    dw0T_h = nc.dram_tensor("dw0T", [128, KC, H], f16, kind="ExternalInput")
    dw1c_h = nc.dram_tensor("dw1c", [128, KC, HC], f16, kind="ExternalInput")
    out_h = nc.dram_tensor("out", [P, 2], f32, kind="ExternalOutput")

    with tile.TileContext(nc) as tc:
        with (
            tc.tile_pool(name="w", bufs=1) as wpool,
            tc.tile_pool(name="s", bufs=1) as spool,
            tc.tile_pool(name="tmp", bufs=2) as tpool,
            tc.tile_pool(name="psum", bufs=1, space="PSUM") as ppool,
        ):
            smallt = wpool.tile([128, NSMALL], f32, tag="small")
            ew1t = wpool.tile([128, KC, H], f16, tag="ew1")
            dw0t = wpool.tile([128, KC, H], f16, tag="dw0")
            dw1t = wpool.tile([128, KC, HC], f16, tag="dw1")

            h1s = spool.tile([128, KC], f16, tag="h1")
            h2s = spool.tile([128, KC], f16, tag="h2")
            rp0s = spool.tile([128, KC], f32, tag="rp0")
            Ast = spool.tile([128, KC, P], f32, tag="Ast")
            gt = spool.tile([128, KC, P], f16, tag="gt")
            pre2b = spool.tile([128, P], f32, tag="pre2b")

            # ---- DMAs: ew1 slabs first, then dw1c, then dw0 slabs ----
            CPS = KC // NSLAB  # chunks per slab
            nc.sync.dma_start(out=ew1t[:, 0:CPS, :], in_=ew1T_h.ap()[:, 0:CPS, :])
            nc.sync.dma_start(out=smallt[:, :], in_=small_h.ap())
            for s in range(1, NSLAB):
                sl = slice(s * CPS, (s + 1) * CPS)
                nc.sync.dma_start(out=ew1t[:, sl, :], in_=ew1T_h.ap()[:, sl, :])
            lo = 0
            for w in (3, 3, 1, 1):
                sl = slice(lo, lo + w)
                nc.sync.dma_start(out=dw0t[:, sl, :], in_=dw0T_h.ap()[:, sl, :])
                lo += w
            nc.sync.dma_start(out=dw1t[:, :, :], in_=dw1c_h.ap())

            # ---- A-scan: a_0 = 1, a_t = max(u0 * a_{t-1} + 1, 0) ----
            # (only depends on the small pack -> runs during the weight DMA)
            u0 = smallt[:, C_U0:C_U0 + KC]
            nc.vector.memset(Ast[:, :, 0], 1.0)
            for t in range(1, P):
                atmp = tpool.tile([128, KC], f32, tag="atmp")
                nc.vector.tensor_tensor(atmp[:, :], Ast[:, :, t - 1], u0, mult)
                nc.vector.tensor_scalar(Ast[:, :, t], atmp[:, :], 1.0, 0.0,
                                        add, amax)

            # ---- h1 = relu(w0c0*v0 + w0c1*v1 + enc_b0) on DVE ----
            t1 = tpool.tile([128, KC], f32, tag="h1a")
            t2 = tpool.tile([128, KC], f32, tag="h1b")
            nc.vector.tensor_scalar(t1[:, :], smallt[:, C_W00:C_W00 + KC],
                                    smallt[:, C_V0:C_V0 + 1], None, mult)
            nc.vector.tensor_scalar(t2[:, :], smallt[:, C_W01:C_W01 + KC],
                                    smallt[:, C_V1:C_V1 + 1], None, mult)
            nc.vector.tensor_tensor(t1[:, :], t1[:, :], t2[:, :], add)
            nc.vector.tensor_tensor(t1[:, :], t1[:, :],
                                    smallt[:, C_EB0:C_EB0 + KC], add)
            nc.vector.tensor_scalar(h1s[:, :], t1[:, :], 0.0, None, amax)

            # ---- h2 = relu(enc_w1 @ h1 + enc_b1) ----
            # kc-outer with 7 parallel accumulators + second pass for chunk 7,
            # so the last-arriving weight slab gates only ~8 matmul pairs
            NACC = KC - 1
            pms = [ppool.tile([128, 1], f32, tag="mv", bufs=7, name=f"pm{i}")
                   for i in range(NACC)]
            for kc in range(KC):
                for mc in range(NACC):
                    nc.tensor.matmul(pms[mc][:, :],
                                     ew1t[:, kc, mc * 128:(mc + 1) * 128],
                                     h1s[:, kc:kc + 1],
                                     start=(kc == 0), stop=(kc == KC - 1))
            pm7 = ppool.tile([128, 1], f32, tag="pp")
            for kc in range(KC):
                nc.tensor.matmul(pm7[:, :],
                                 ew1t[:, kc, NACC * 128:KC * 128],
                                 h1s[:, kc:kc + 1],
                                 start=(kc == 0), stop=(kc == KC - 1))
            for mc in range(KC):
                pm = pm7 if mc == NACC else pms[mc]
                nc.scalar.activation(h2s[:, mc:mc + 1], pm[:, :], Relu,
                                     bias=smallt[:, C_EB1 + mc:C_EB1 + mc + 1])

            # ---- rp0 = relu(dec_w0 @ h2 + dec_b0); G^T = rp0 * A ----
            # kc-outer so the last-arriving dec_w0 slab gates few matmuls
            pjs = [ppool.tile([128, 1], f32, tag="mv", bufs=7, name=f"pj{i}")
                   for i in range(NACC)]
            for kc in range(KC):
                for jc in range(NACC):
                    nc.tensor.matmul(pjs[jc][:, :],
                                     dw0t[:, kc, jc * 128:(jc + 1) * 128],
                                     h2s[:, kc:kc + 1],
                                     start=(kc == 0), stop=(kc == KC - 1))
            pj7 = ppool.tile([128, 1], f32, tag="pp")
            for kc in range(KC):
                nc.tensor.matmul(pj7[:, :],
                                 dw0t[:, kc, NACC * 128:KC * 128],
                                 h2s[:, kc:kc + 1],
                                 start=(kc == 0), stop=(kc == KC - 1))
            for jc in range(KC):
                pj = pj7 if jc == NACC else pjs[jc]
                nc.scalar.activation(rp0s[:, jc:jc + 1], pj[:, :], Relu,
                                     bias=smallt[:, C_DB0 + jc:C_DB0 + jc + 1])
            nc.vector.tensor_tensor(gt[:, :, :], Ast[:, :, :],
                                    rp0s[:, :].broadcast_to([128, KC, P]),
                                    mult)

            # ---- pre2^T = dec_w1_shard @ G + dec_b1_shard  (128 j, 20 t) ----
            pp = ppool.tile([128, P], f32, tag="pp")
            for kc in range(KC):
                nc.tensor.matmul(pp[:, :], dw1t[:, kc, :], gt[:, kc, :],
                                 start=(kc == 0), stop=(kc == KC - 1))
            nc.vector.tensor_scalar(pre2b[:, :], pp[:, :],
                                    smallt[:, C_DB1:C_DB1 + 1], None, add)

            # ---- scan2: o_t = relu(pre2_t + u1 * o_{t-1}) via two HW scans
            # with the shift g_t = u*g_{t-1} - c_t:
            #   q_t = max(u*q_{t-1}, g_t)  and  o_t = q_t - g_t  (exact)
            u1b = smallt[:, C_U1:C_U1 + 1].broadcast_to([HC, P])
            gam = spool.tile([HC, P], f32, tag="gam")
            qsc = spool.tile([HC, P], f32, tag="qsc")
            nc.vector.tensor_tensor_scan(gam[:, :], u1b, pre2b[:, :], 0.0,
                                         mult, sub)
            nc.vector.tensor_tensor_scan(qsc[:, :], u1b, gam[:, :], 0.0,
                                         mult, amax)

            # ---- head partial: (20,2) = q.T @ ow - gam.T @ ow  (o = q-gam)
            hp = ppool.tile([P, 2], f32, tag="pp")
            nc.tensor.matmul(hp[:, :], qsc[:, :], smallt[:, C_OW:C_OW + 2],
                             start=True, stop=False)
            nc.tensor.matmul(hp[:, :], gam[:, :], smallt[:, C_OWN:C_OWN + 2],
                             start=False, stop=True)
            outs = spool.tile([P, 2], f32, tag="outs")
            nc.vector.tensor_copy(outs[:, :], hp[:, :])
            nc.sync.dma_start(out=out_h.ap(), in_=outs[:, :])

    nc.compile()
    return nc


def _chunk_major(vec):
    # vec (1024,) -> (128, 8) with [p, c] = vec[c*128 + p]
    return np.ascontiguousarray(vec.reshape(KC, 128).T)


def _tile_f16(wT):
    # W.T (1024, m) f32 -> (128, KC, m) f16 with [p, kc, m] = W.T[kc*128+p, m]
    return np.ascontiguousarray(
        wT.astype(np.float16).reshape(KC, 128, wT.shape[1]).transpose(1, 0, 2))


def _kernel_replicated(x, enc_w0, enc_u0, enc_b0, enc_w1, enc_u1, enc_b1,
                       dec_w0, dec_u0, dec_b0, dec_w1, dec_u1, dec_b1,
                       out_w, out_b):
    import os
    from concourse.bass_utils import run_bass_kernel_spmd

    if "nc" not in _CACHE:
        _CACHE["nc"] = _build()
    nc = _CACHE["nc"]

    f = np.float32
    v = np.asarray(x, f)[0, -1, :]                              # (2,)
    ew0 = np.asarray(enc_w0, f)                                 # (1024, 2)
    ew1tiled = _tile_f16(np.asarray(enc_w1, f).T)               # (128, 8, 1024)
    dw0tiled = _tile_f16(np.asarray(dec_w0, f).T)               # (128, 8, 1024)
    dw1T = np.asarray(dec_w1, f).T                              # (1024, 1024)
    owT = np.asarray(out_w, f).T                                # (1024, 2)

    base = np.zeros((128, NSMALL), f)
    base[:, C_U0:C_U0 + KC] = _chunk_major(np.asarray(dec_u0, f))
    base[:, C_EB1:C_EB1 + KC] = _chunk_major(np.asarray(enc_b1, f))
    base[:, C_DB0:C_DB0 + KC] = _chunk_major(np.asarray(dec_b0, f))
    base[:, C_EB0:C_EB0 + KC] = _chunk_major(np.asarray(enc_b0, f))
    base[:, C_W00:C_W00 + KC] = _chunk_major(np.ascontiguousarray(ew0[:, 0]))
    base[:, C_W01:C_W01 + KC] = _chunk_major(np.ascontiguousarray(ew0[:, 1]))
    base[:, C_V0] = v[0]
    base[:, C_V1] = v[1]

    in_maps = []
    for c in range(NCORES):
        jsl = slice(c * HC, (c + 1) * HC)
        small = base.copy()
        small[:, C_U1] = np.asarray(dec_u1, f)[jsl]
        small[:, C_DB1] = np.asarray(dec_b1, f)[jsl]
        small[:, C_OW:C_OW + 2] = owT[jsl, :]
        small[:, C_OWN:C_OWN + 2] = -owT[jsl, :]
        in_maps.append({
            "small": small,
            "ew1T": ew1tiled,
            "dw0T": dw0tiled,
            "dw1c": _tile_f16(np.ascontiguousarray(dw1T[:, jsl])),
        })

    trace = bool(os.environ.get("KERNEL_TRACE"))
    res = run_bass_kernel_spmd(nc, in_maps, core_ids=list(range(NCORES)),
                               trace=trace)
    _CACHE["last_result"] = res
    partials = [res.results[c]["out"] for c in range(NCORES)]
    return (np.sum(partials, axis=0) + np.asarray(out_b, f)).astype(f)



# ---- sharded-path small-pack columns (distinct from the replicated pack) ----
S_C_U0S = 0
S_C_EB0 = 8
S_C_W00 = 16
S_C_W01 = 24
S_C_V0 = 32
S_C_V1 = 33
S_C_EB1 = 34
S_C_DB0 = 35
S_C_U1 = 36
S_C_DB1 = 37
S_C_OW = 38
S_C_OWN = 40
S_NSMALL = 42

def _rdests_for(d):
    # single real destination (delta_rid=0, delta_tpb=d) at slot position d;
    # positions with bit 2 set land on D2D-capable engines, matching bit 2 of
    # the tpb delta, which is what the mask validator requires.
    r = [None] * 8
    r[d] = (0, d)
    return r


class _seed_sched_sems:
    """Tile's scheduling pass runs a single-core no-exec CoreSim in which
    remotely-incremented semaphores never advance, so author-placed waits on
    them deadlock the scheduler.  While active, pre-seed the given sems to a
    large value in every CoreSim before it simulates.  The scheduling sim
    starts at the tile block (start_pc), not the function preamble, so the
    seeds are not wiped by the kernel-entry sem_clear.  Only the build-time
    scheduling pass runs under this context; functional/timing simulation and
    hardware execution are unaffected."""

    def __init__(self, sems):
        self.sems = sems

    def __enter__(self):
        import concourse.mybir as mb
        from concourse.bass_interp import CoreSim

        self._cls = CoreSim
        self._orig = CoreSim.simulate
        sems = self.sems

        def patched(sim, *a, **k):
            for s in sems:
                sim.update_semaphore(mb.SyncUpdate(
                    sync_type="semaphore", id=s.num, ant_name=s.name,
                    update_mode="sem-add-imm", update_value=1 << 12))
            return self._orig(sim, *a, **k)

        CoreSim.simulate = patched
        return self

    def __exit__(self, *exc):
        self._cls.simulate = self._orig
        return False


def _build_probe():
    """Tiny SPMD kernel: all-gather each core's id via the XOR exchange.

    Slots are padded to 32 bytes (8 f32) -- sub-32B remote-DMA payloads
    showed partial-delivery corruption on hardware.  out slots[:, s*8] = id of
    the core whose chunk lands in slot s on this core.  Exercises desc-gen,
    trigger and remote semaphores exactly like the main kernel's exchanges.
    """
    import concourse.mybir as mybir
    from concourse import bacc, tile
    import bass_rust

    f32 = mybir.dt.float32
    W = 8
    # detect_race_conditions=False: the functional simulator's RDMA race
    # checker attributes a deferred send to the desc-gen instruction's
    # semaphore watermark (recorded before the trigger's recv-gate wait), so
    # the correct cumulative-threshold protocol here trips it spuriously.
    nc = bacc.Bacc("TRN2", target_bir_lowering=False, debug=False,
                   num_devices=NCORES, num_swdge_queues=1,
                   detect_race_conditions=False)
    myid_h = nc.dram_tensor("myid", [128, W], f32, kind="ExternalInput")
    out_h = nc.dram_tensor("slots", [128, KC * W], f32, kind="ExternalOutput")
    gsem = nc.alloc_semaphore("gsem")
    lsem = nc.alloc_semaphore("lsem")
    dep = bass_rust.add_dep_helper

    with _seed_sched_sems([gsem]), tile.TileContext(nc) as tc:
        with tc.tile_pool(name="s", bufs=1) as spool:
            buf = spool.tile([128, KC * W], f32, tag="buf")
            nc.sync.dma_start(out=buf[:, 0:W], in_=myid_h.ap())
            preps = []
            for r in range(3):
                d = 1 << r
                pr = nc.gpsimd.remote_dma_broadcast(
                    out_ap=buf[:, d * W:2 * d * W], in_ap=buf[:, 0:d * W],
                    remote_sem=gsem, local_sem=lsem,
                    rdests=_rdests_for(d), queue_num=0)
                if preps:
                    dep(pr.ins, preps[-1].ins, reason="fifo order")
                preps.append(pr)
            trigs = []
            for r in range(3):
                if r > 0:
                    # Pool is in-order; the wait can only be satisfied after
                    # OUR previous round fired (symmetric SPMD).
                    w = nc.gpsimd.wait_ge(gsem, 2 * r)
                    dep(w.ins, trigs[-1].ins, reason="pool order")
                tr = nc.gpsimd.trigger_dma(1, queue_num=0)
                if r > 0:
                    dep(tr.ins, w.ins, reason="recv gate")
                    dep(tr.ins, preps[r].ins, reason="descs written")
                    dep(tr.ins, trigs[-1].ins, reason="fifo trigger order")
                trigs.append(tr)
            wv = nc.vector.wait_ge(gsem, 6)
            dep(wv.ins, trigs[-1].ins, reason="after own last trigger")
            outb = spool.tile([128, KC * W], f32, tag="outb", name="outb")
            cp = nc.vector.tensor_copy(outb[:, :], buf[:, :])
            dep(cp.ins, wv.ins, reason="gather complete")
            nc.sync.dma_start(out=out_h.ap(), in_=outb[:, :])
    nc.compile()
    return nc


def _build_sharded():
    import concourse.mybir as mybir
    from concourse import bacc, tile
    import bass_rust

    f32 = mybir.dt.float32
    f16 = mybir.dt.float16
    Relu = mybir.ActivationFunctionType.Relu
    mult = mybir.AluOpType.mult
    add = mybir.AluOpType.add
    sub = mybir.AluOpType.subtract
    amax = mybir.AluOpType.max
    dep = bass_rust.add_dep_helper

    nc = bacc.Bacc("TRN2", target_bir_lowering=False, debug=False,
                   num_devices=NCORES, num_swdge_queues=2,
                   detect_race_conditions=False)

    small_h = nc.dram_tensor("small", [128, S_NSMALL], f32, kind="ExternalInput")
    # row-shards, pre-transposed+tiled on host: [p, s, m]
    ew1_h = nc.dram_tensor("ew1c", [128, KC, HC], f16, kind="ExternalInput")
    dw0_h = nc.dram_tensor("dw0c", [128, KC, HC], f16, kind="ExternalInput")
    dw1_h = nc.dram_tensor("dw1c", [128, KC, HC], f16, kind="ExternalInput")
    out_h = nc.dram_tensor("out", [P, 2], f32, kind="ExternalOutput")

    h2sem = nc.alloc_semaphore("h2sem")
    rpsem = nc.alloc_semaphore("rpsem")
    lsem0 = nc.alloc_semaphore("rdma_lsem0")
    lsem1 = nc.alloc_semaphore("rdma_lsem1")

    with _seed_sched_sems([h2sem, rpsem]), tile.TileContext(nc) as tc:
        with (
            tc.tile_pool(name="w", bufs=1) as wpool,
            tc.tile_pool(name="s", bufs=1) as spool,
            tc.tile_pool(name="tmp", bufs=2) as tpool,
            tc.tile_pool(name="psum", bufs=1, space="PSUM") as ppool,
        ):
            smallt = wpool.tile([128, S_NSMALL], f32, tag="small")
            ew1t = wpool.tile([128, KC, HC], f16, tag="ew1")
            dw0t = wpool.tile([128, KC, HC], f16, tag="dw0")
            dw1t = wpool.tile([128, KC, HC], f16, tag="dw1")

            h1s = spool.tile([128, KC], f16, tag="h1")
            h2all = spool.tile([128, KC], f16, tag="h2all")
            rpall = spool.tile([128, KC], f32, tag="rpall")
            ucoef = spool.tile([128, KC, P], f32, tag="ucoef")
            onesc = spool.tile([128, 1], f32, tag="onesc")
            gsc = spool.tile([128, KC, P], f32, tag="gsc")
            qsc0 = spool.tile([128, KC, P], f32, tag="qsc0")
            Ast = spool.tile([128, KC, P], f32, tag="Ast")
            gt = spool.tile([128, KC, P], f16, tag="gt")
            pre2b = spool.tile([128, P], f32, tag="pre2b")
            gam = spool.tile([HC, P], f32, tag="gam")
            qsc = spool.tile([HC, P], f32, tag="qsc")
            outs = spool.tile([P, 2], f32, tag="outs")

            # ---- DMAs: small first (feeds all early DVE work), then the
            # three 256KB weight shards; split across the two HWDGE engines ----
            nc.sync.dma_start(out=smallt[:, :], in_=small_h.ap())
            nc.sync.dma_start(out=ew1t[:, :, :], in_=ew1_h.ap())
            nc.scalar.dma_start(out=dw0t[:, :, :], in_=dw0_h.ap())
            nc.scalar.dma_start(out=dw1t[:, :, :], in_=dw1_h.ap())

            # ---- Pool: pre-generate all exchange descriptors (deferred
            # source reads; fired by trigger_dma later).  h2 rounds on q0,
            # rp0 rounds on q1. ----
            h2preps, rppreps = [], []
            for r in range(3):
                d = 1 << r
                pr = nc.gpsimd.remote_dma_broadcast(
                    out_ap=h2all[:, d:2 * d], in_ap=h2all[:, 0:d],
                    remote_sem=h2sem, local_sem=lsem0,
                    rdests=_rdests_for(d), queue_num=0)
                if h2preps:
                    dep(pr.ins, h2preps[-1].ins, reason="fifo order q0")
                h2preps.append(pr)
            for r in range(3):
                d = 1 << r
                pr = nc.gpsimd.remote_dma_broadcast(
                    out_ap=rpall[:, d:2 * d], in_ap=rpall[:, 0:d],
                    remote_sem=rpsem, local_sem=lsem1,
                    rdests=_rdests_for(d), queue_num=1)
                dep(pr.ins, (rppreps[-1] if rppreps else h2preps[-1]).ins,
                    reason="pool desc-gen order")
                rppreps.append(pr)

            # ---- DVE early work (only needs the small pack) ----
            # h1 = relu(w0c0*v0 + w0c1*v1 + enc_b0)
            t1 = tpool.tile([128, KC], f32, tag="h1a")
            t2 = tpool.tile([128, KC], f32, tag="h1b")
            nc.vector.tensor_scalar(t1[:, :], smallt[:, S_C_W00:S_C_W00 + KC],
                                    smallt[:, S_C_V0:S_C_V0 + 1], None, mult)
            nc.vector.tensor_scalar(t2[:, :], smallt[:, S_C_W01:S_C_W01 + KC],
                                    smallt[:, S_C_V1:S_C_V1 + 1], None, mult)
            nc.vector.tensor_tensor(t1[:, :], t1[:, :], t2[:, :], add)
            nc.vector.tensor_tensor(t1[:, :], t1[:, :],
                                    smallt[:, S_C_EB0:S_C_EB0 + KC], add)
            h1_last = nc.vector.tensor_scalar(h1s[:, :], t1[:, :], 0.0, None,
                                              amax)

            # A-scan via two HW scans with zeroed chunk-boundary coefficients:
            #   gsc_t = u_t*gsc_{t-1} - 1,  q_t = max(u_t*q_{t-1}, gsc_t),
            #   A = q - gsc     (u_t = u0 lane coef, 0 at each chunk's t=0)
            nc.vector.tensor_copy(
                ucoef[:, :, :],
                smallt[:, S_C_U0S:S_C_U0S + KC].broadcast_to([128, KC, P]))
            nc.vector.memset(ucoef[:, :, 0:1], 0.0)
            nc.vector.memset(onesc[:, :], 1.0)
            flat = "p a b -> p (a b)"
            nc.vector.tensor_tensor_scan(
                gsc[:, :, :].rearrange(flat), ucoef[:, :, :].rearrange(flat),
                onesc[:, :].broadcast_to([128, KC * P]), 0.0, mult, sub)
            nc.vector.tensor_tensor_scan(
                qsc0[:, :, :].rearrange(flat), ucoef[:, :, :].rearrange(flat),
                gsc[:, :, :].rearrange(flat), 0.0, mult, amax)
            ast_op = nc.vector.tensor_tensor(Ast[:, :, :], qsc0[:, :, :],
                                             gsc[:, :, :], sub)

            # ---- h2_c = relu(ew1_shard @ h1 + eb1_c) -> h2all slot 0 ----
            pmh = ppool.tile([128, 1], f32, tag="pmh")
            pmh_mms = []
            for kc in range(KC):
                pmh_mms.append(nc.tensor.matmul(
                    pmh[:, :], ew1t[:, kc, :], h1s[:, kc:kc + 1],
                    start=(kc == 0), stop=(kc == KC - 1)))
            act_h2 = nc.scalar.activation(h2all[:, 0:1], pmh[:, :], Relu,
                                          bias=smallt[:, S_C_EB1:S_C_EB1 + 1])

            # ---- h2 all-gather: 3 XOR rounds on q0 ----
            # Pool is in-order: every wait must sit after the local trigger it
            # transitively needs (symmetric SPMD), or the core deadlocks.
            h2trigs = []
            for r in range(3):
                if r > 0:
                    w = nc.gpsimd.wait_ge(h2sem, 2 * r)
                    dep(w.ins, h2trigs[-1].ins, reason="pool order")
                tr = nc.gpsimd.trigger_dma(1, queue_num=0)
                if r > 0:
                    dep(tr.ins, w.ins, reason="recv gate")
                    dep(tr.ins, h2preps[r].ins, reason="descs written")
                    dep(tr.ins, h2trigs[-1].ins, reason="fifo trigger order")
                else:
                    # the preps predate the producer in program order, so the
                    # deferred-source dep was never recorded; attach it here.
                    dep(tr.ins, act_h2.ins, reason="h2 slot0 ready")
                h2trigs.append(tr)

            # ---- p0_c = dw0_shard @ h2 (all slots); rp0_c -> rpall slot 0 ----
            # PE is in-order too: the wait must come after the h2 matvec.
            wpe = nc.tensor.wait_ge(h2sem, 6)
            dep(wpe.ins, pmh_mms[-1].ins, reason="pe order: wait after h2 mm")
            pmp_mms = []
            pmp = ppool.tile([128, 1], f32, tag="pmp")
            for s in range(KC):
                mm = nc.tensor.matmul(pmp[:, :], dw0t[:, s, :],
                                      h2all[:, s:s + 1],
                                      start=(s == 0), stop=(s == KC - 1))
                if s == 0:
                    dep(mm.ins, wpe.ins, reason="h2 gather complete")
                pmp_mms.append(mm)
            act_rp = nc.scalar.activation(rpall[:, 0:1], pmp[:, :], Relu,
                                          bias=smallt[:, S_C_DB0:S_C_DB0 + 1])

            # ---- rp0 all-gather: 3 XOR rounds on q1 ----
            rptrigs = []
            for r in range(3):
                if r > 0:
                    w = nc.gpsimd.wait_ge(rpsem, 2 * r)
                    dep(w.ins, rptrigs[-1].ins, reason="pool order")
                tr = nc.gpsimd.trigger_dma(1, queue_num=1)
                if r > 0:
                    dep(tr.ins, w.ins, reason="recv gate")
                    dep(tr.ins, rppreps[r].ins, reason="descs written")
                    dep(tr.ins, rptrigs[-1].ins, reason="fifo trigger order")
                else:
                    # keep the whole exchange sequence in Pool program order
                    dep(tr.ins, h2trigs[-1].ins, reason="pool order")
                    dep(tr.ins, act_rp.ins, reason="rp0 slot0 ready")
                rptrigs.append(tr)

            # ---- G = rp0 * A (slot-aligned); pre2_c = dw1_shard @ G ----
            # DVE in-order: the wait must come after all upstream DVE work
            # (h1 feeds the h2 matvec; Ast feeds the G multiply right after).
            wdv = nc.vector.wait_ge(rpsem, 6)
            dep(wdv.ins, h1_last.ins, reason="dve order: h1 before wait")
            dep(wdv.ins, ast_op.ins, reason="dve order: Ast before wait")
            gm = nc.vector.tensor_tensor(
                gt[:, :, :], Ast[:, :, :],
                rpall[:, :].broadcast_to([128, KC, P]), mult)
            dep(gm.ins, wdv.ins, reason="rp0 gather complete")
            pp = ppool.tile([128, P], f32, tag="pp")
            pre2_mms = []
            for s in range(KC):
                mm = nc.tensor.matmul(pp[:, :], dw1t[:, s, :], gt[:, s, :],
                                      start=(s == 0), stop=(s == KC - 1))
                if s == 0:
                    dep(mm.ins, pmp_mms[-1].ins, reason="pe order")
                pre2_mms.append(mm)
            nc.vector.tensor_scalar(pre2b[:, :], pp[:, :],
                                    smallt[:, S_C_DB1:S_C_DB1 + 1], None, add)

            # ---- scan2 (own 128 lanes), head partial, out ----
            u1b = smallt[:, S_C_U1:S_C_U1 + 1].broadcast_to([HC, P])
            nc.vector.tensor_tensor_scan(gam[:, :], u1b, pre2b[:, :], 0.0,
                                         mult, sub)
            nc.vector.tensor_tensor_scan(qsc[:, :], u1b, gam[:, :], 0.0,
                                         mult, amax)
            hp = ppool.tile([P, 2], f32, tag="hp")
            hm = nc.tensor.matmul(hp[:, :], qsc[:, :], smallt[:, S_C_OW:S_C_OW + 2],
                                  start=True, stop=False)
            dep(hm.ins, pre2_mms[-1].ins, reason="pe order")
            nc.tensor.matmul(hp[:, :], gam[:, :], smallt[:, S_S_C_OWN:S_S_C_OWN + 2],
                             start=False, stop=True)
            nc.vector.tensor_copy(outs[:, :], hp[:, :])
            nc.sync.dma_start(out=out_h.ap(), in_=outs[:, :])

    nc.compile()
    return nc


def _chunk_major(vec):
    # vec (1024,) -> (128, 8) with [p, c] = vec[c*128 + p]
    return np.ascontiguousarray(vec.reshape(KC, 128).T)


def _probe_slot_map():
    """Run the probe kernel (retrying -- remote-DMA delivery is flaky on this
    stack); return slot_map[c][s] = source core of slot s on core c."""
    from concourse.bass_utils import run_bass_kernel_spmd

    W = 8
    if "probe_map" in _CACHE:
        return _CACHE["probe_map"]
    nc = _CACHE.get("probe_nc")
    if nc is None:
        nc = _CACHE["probe_nc"] = _build_probe()
    last_err = None
    for _ in range(3):
        try:
            in_maps = [{"myid": np.full((128, W), c, np.float32)}
                       for c in range(NCORES)]
            res = run_bass_kernel_spmd(nc, in_maps,
                                       core_ids=list(range(NCORES)))
            slot_map = []
            for c in range(NCORES):
                s = np.asarray(res.results[c]["slots"]).reshape(128, KC, W)
                for k in range(KC):
                    assert (s[:, k, :] == s[0, k, 0]).all(), \
                        f"probe: nonuniform slot {k} on core {c}"
                ids = s[0, :, 0].astype(int)
                assert ids[0] == c, f"probe: slot0 != self on core {c}: {ids}"
                assert sorted(ids.tolist()) == list(range(NCORES)), \
                    f"probe: not a permutation on core {c}: {ids}"
                slot_map.append(ids.tolist())
            _CACHE["probe_map"] = slot_map
            return slot_map
        except Exception as e:
            last_err = e
    raise RuntimeError(f"probe failed: {last_err}")


def _shard_tiled_f16(w, c, chunks):
    # rows shard c of W (1024x1024), k-chunks ordered per `chunks`:
    # out[p, s, m] = W[c*128 + m, chunks[s]*128 + p]  (f16, contiguous)
    rows = np.asarray(w, np.float32)[c * HC:(c + 1) * HC, :]  # (128, 1024)
    rt = rows.astype(np.float16).T.reshape(KC, 128, HC)       # [kc, p, m]
    return np.ascontiguousarray(rt[chunks].transpose(1, 0, 2))  # [p, s, m]


def _sharded_in_maps(inputs, slot_map):
    f = np.float32
    x, enc_w0, enc_b0 = inputs["x"], inputs["enc_w0"], inputs["enc_b0"]
    v = np.asarray(x, f)[0, -1, :]
    ew0 = np.asarray(enc_w0, f)

    base = np.zeros((128, S_NSMALL), f)
    base[:, S_C_EB0:S_C_EB0 + KC] = _chunk_major(np.asarray(enc_b0, f))
    base[:, S_C_W00:S_C_W00 + KC] = _chunk_major(np.ascontiguousarray(ew0[:, 0]))
    base[:, S_C_W01:S_C_W01 + KC] = _chunk_major(np.ascontiguousarray(ew0[:, 1]))
    base[:, S_C_V0] = v[0]
    base[:, S_C_V1] = v[1]

    u0 = np.asarray(inputs["dec_u0"], f)
    owT = np.asarray(inputs["out_w"], f).T
    in_maps = []
    for c in range(NCORES):
        chunks = slot_map[c]
        jsl = slice(c * HC, (c + 1) * HC)
        small = base.copy()
        for s in range(KC):
            j = chunks[s]
            small[:, S_C_U0S + s] = u0[j * HC:(j + 1) * HC]
        small[:, S_C_EB1] = np.asarray(inputs["enc_b1"], f)[jsl]
        small[:, S_C_DB0] = np.asarray(inputs["dec_b0"], f)[jsl]
        small[:, S_C_U1] = np.asarray(inputs["dec_u1"], f)[jsl]
        small[:, S_C_DB1] = np.asarray(inputs["dec_b1"], f)[jsl]
        small[:, S_C_OW:S_C_OW + 2] = owT[jsl, :]
        small[:, S_S_C_OWN:S_S_C_OWN + 2] = -owT[jsl, :]
        in_maps.append({
            "small": small,
            "ew1c": _shard_tiled_f16(inputs["enc_w1"], c, list(range(KC))),
            "dw0c": _shard_tiled_f16(inputs["dec_w0"], c, chunks),
            "dw1c": _shard_tiled_f16(inputs["dec_w1"], c, chunks),
        })
    return in_maps




def _host_expected(inputs):
    """fp32 numpy evaluation of the reduced network -- used only to DETECT
    hardware data corruption in the sharded path (flaky remote-DMA delivery
    observed); the returned tensor is always the device result."""
    f = np.float32
    v = np.asarray(inputs["x"], f)[0, -1, :]
    h1 = np.maximum(np.asarray(inputs["enc_w0"], f) @ v
                    + np.asarray(inputs["enc_b0"], f), 0)
    h2 = np.maximum(np.asarray(inputs["enc_w1"], f) @ h1
                    + np.asarray(inputs["enc_b1"], f), 0)
    p0 = np.asarray(inputs["dec_w0"], f) @ h2 + np.asarray(inputs["dec_b0"], f)
    u0 = np.asarray(inputs["dec_u0"], f)
    g = np.zeros_like(p0)
    G = np.empty((H, P), f)
    for t in range(P):
        g = np.maximum(p0 + u0 * g, 0)
        G[:, t] = g
    pre2 = np.asarray(inputs["dec_w1"], f) @ G \
        + np.asarray(inputs["dec_b1"], f)[:, None]
    u1 = np.asarray(inputs["dec_u1"], f)
    o = np.zeros(H, f)
    O = np.empty((H, P), f)
    for t in range(P):
        o = np.maximum(pre2[:, t] + u1 * o, 0)
        O[:, t] = o
    return O.T @ np.asarray(inputs["out_w"], f).T


_CHILD_CODE = """
import sys, numpy as np
import importlib.util
spec = importlib.util.spec_from_file_location("kmod", sys.argv[1])
kmod = importlib.util.module_from_spec(spec)
spec.loader.exec_module(kmod)
data = np.load(sys.argv[2])
inputs = {k: data[k] for k in data.files}
from concourse.bass_utils import run_bass_kernel_spmd
slot_map = kmod._probe_slot_map()
nc = kmod._build_sharded()
in_maps = kmod._sharded_in_maps(inputs, slot_map)
res = run_bass_kernel_spmd(nc, in_maps, core_ids=list(range(kmod.NCORES)))
partials = [res.results[c]["out"] for c in range(kmod.NCORES)]
np.save(sys.argv[3], np.sum(partials, axis=0).astype(np.float32))
"""


def _try_sharded_subprocess(inputs):
    """Run probe + sharded kernel in a child process (a flaky remote-DMA run
    can take down the NRT client; isolation keeps this process healthy).
    Returns the summed head partials (before +out_b) or None."""
    import os
    import subprocess
    import sys
    import tempfile

    with tempfile.TemporaryDirectory() as td:
        inp = os.path.join(td, "in.npz")
        outp = os.path.join(td, "out.npy")
        np.savez(inp, **{k: np.asarray(v) for k, v in inputs.items()})
        try:
            r = subprocess.run(
                [sys.executable, "-c", _CHILD_CODE, __file__, inp, outp],
                timeout=600, capture_output=True)
            if r.returncode != 0 or not os.path.exists(outp):
                return None
            return np.load(outp)
        except Exception:
            return None


def kernel(x, enc_w0, enc_u0, enc_b0, enc_w1, enc_u1, enc_b1,
           dec_w0, dec_u0, dec_b0, dec_w1, dec_u1, dec_b1,
           out_w, out_b):
    inputs = dict(x=x, enc_w0=enc_w0, enc_u0=enc_u0, enc_b0=enc_b0,
                  enc_w1=enc_w1, enc_u1=enc_u1, enc_b1=enc_b1,
                  dec_w0=dec_w0, dec_u0=dec_u0, dec_b0=dec_b0,
                  dec_w1=dec_w1, dec_u1=dec_u1, dec_b1=dec_b1,
                  out_w=out_w, out_b=out_b)
    ob = np.asarray(out_b, np.float32)
    expect = _host_expected(inputs)
    scale = max(float(np.abs(expect).max()), 1e-30)
    for _ in range(2):
        part = _try_sharded_subprocess(inputs)
        if part is None:
            continue
        if np.abs(part + ob - (expect + ob)).max() / scale < 5e-3:
            _CACHE["path"] = "sharded"
            return (part + ob).astype(np.float32)
    _CACHE["path"] = "replicated"
    return _kernel_replicated(**inputs)
